# revision 27
# baseline (speedup 1.0000x reference)
"""Trainium2 Bass kernel: dimensional (channel) attention token-mixing block.

Computation (per batch b):
    xt = x[b].T                               # [C, N]
    q  = xt @ wq.T                            # [C, N]   (heads: N = H*NH)
    kv = xt @ wkv.T                           # [C, 2N]
    q, k normalized over NH per (c, head)
    kt[h] = sum_c k_hat[c,h,:] * v[c,h,:] * scale[h]     # [NH]
    o  = gelu(q_hat * kt)                     # [C, N]
    out[b] = (o @ wproj.T + bproj).T          # [N, C]

Sharding: data-parallel over B across 8 cores (2 batches/core), weights
replicated.  All tensors kept in "transposed" [n, c] layout on device so the
contraction dim n always lies on SBUF partitions.

Precision strategy: all big matmuls run as fp8(e4m3) DoubleRow pairs with
residual compensation.  Each operand A is split A = A_hi + A_lo/16 with
A_hi = fp8(A), A_lo = fp8(16*(A - A_hi)).  Then
    A@W = [A_hi@W_hi] + ([A_lo@W_hi] + [A_hi@W_lo]) / 16   (lo*lo dropped)
The hi*hi product accumulates in one PSUM bank (P0), both cross terms in a
second (P1); the eviction to SBUF computes P0 + P1/16 in a single
scalar_tensor_tensor op.  DoubleRow contracts 256 rows per instruction at
0.5 cycles/output-row, so the 3-term compensated matmul runs 1.33x faster
than bf16 with slightly BETTER accuracy (residuals capture bf16-level
precision).

Scale folding (weights stored x64 so fp8 sees ~N(0,1) values):
  - q_sb/k_sb = 64*true: killed by the q/k normalization (rnorm computed
    from ssq*2^-12 gives exactly 1/||true||).
  - v_sb = 64*true and k_sb*rnorm = 64*khat: the host folds 1/(64*4096)
    into the per-head `scale` tensor.
  - o is stored as fp8 pair at 64*true (o_hi = fp8(64*gelu),
    o_lo = fp8(1024*residual)); wproj x64: the host folds the 2^-12
    descale + bias into the final tensor_scalar (bias pre-scaled x4096).
"""

import sys

import numpy as np
import ml_dtypes

if "/opt/trn_rl_repo" not in sys.path:
    sys.path.insert(0, "/opt/trn_rl_repo")

import concourse.bass as bass
import concourse.bacc as bacc
import concourse.mybir as mybir
import concourse.tile as tile
from concourse.bass_utils import run_bass_kernel_spmd

B, N, C, H = 16, 4096, 384, 8
NH = N // H          # 512
P = 128
NT = N // P          # 32 contraction chunks of 128
NP = NT // 2         # 16 DoubleRow k-tile pairs
MSUB = NH // P       # 4 output row-subtiles per head
NCORES = 8
BPC = B // NCORES    # batches per core

dt = mybir.dt
AF = mybir.ActivationFunctionType
ALU = mybir.AluOpType
PM = mybir.MatmulPerfMode
F8 = ml_dtypes.float8_e4m3
BF16 = ml_dtypes.bfloat16

WSCALE = 64.0        # weights stored x64 (fp8 normal range)
RSCALE = 16.0        # residual (lo) tensors stored x16
OSCALE = 64.0        # gelu output stored x64

_NC = None
_LAST_RESULTS = None


def _build_nc(act_fn=None):
    if act_fn is None:
        act_fn = AF.Gelu
    nc = bacc.Bacc("TRN2", target_bir_lowering=False, debug=False)

    xh_d = nc.dram_tensor("x_hi", [P, NT, BPC, C], dt.float8e4, kind="ExternalInput")
    xl_d = nc.dram_tensor("x_lo", [P, NT, BPC, C], dt.float8e4, kind="ExternalInput")
    wqh_d = nc.dram_tensor("wq_hi", [NT, P, NT, P], dt.float8e4, kind="ExternalInput")
    wql_d = nc.dram_tensor("wq_lo", [NT, P, NT, P], dt.float8e4, kind="ExternalInput")
    wkvh_d = nc.dram_tensor("wkv_hi", [2 * NT, P, NT, P], dt.float8e4, kind="ExternalInput")
    wkvl_d = nc.dram_tensor("wkv_lo", [2 * NT, P, NT, P], dt.float8e4, kind="ExternalInput")
    wph_d = nc.dram_tensor("wproj_hi", [NT, P, NT, P], dt.float8e4, kind="ExternalInput")
    wpl_d = nc.dram_tensor("wproj_lo", [NT, P, NT, P], dt.float8e4, kind="ExternalInput")
    bias_d = nc.dram_tensor("bias", [P, NT], dt.float32, kind="ExternalInput")
    # col h: unused; col H+h: (scale[h] * 2^-18)^2, folded into rn_k via the
    # Sqrt activation's input scale.
    scale_d = nc.dram_tensor("scale", [P, 2 * H], dt.float32, kind="ExternalInput")
    out_d = nc.dram_tensor("out", [BPC, NT, P, C], dt.float32, kind="ExternalOutput")

    with tile.TileContext(nc) as tc:
        with (
            tc.tile_pool(name="const", bufs=1) as cpool,
            tc.tile_pool(name="wts", bufs=8) as wpool,
            tc.tile_pool(name="qkv", bufs=1) as qkvpool,
            tc.tile_pool(name="sqp", bufs=4) as sqpool,
            tc.tile_pool(name="nrm", bufs=2) as nrmpool,
            tc.tile_pool(name="scr", bufs=4) as scrpool,
            tc.tile_pool(name="outp", bufs=4) as outpool,
            tc.tile_pool(name="mmps", bufs=6, space="PSUM") as mmpsum,
            tc.tile_pool(name="ssqps", bufs=2, space="PSUM") as ssqpsum,
        ):
            xh_sb = cpool.tile([P, NT, BPC, C], dt.float8e4)
            xl_sb = cpool.tile([P, NT, BPC, C], dt.float8e4)
            oh_sb = cpool.tile([P, NT, BPC, C], dt.float8e4)
            ol_sb = cpool.tile([P, NT, BPC, C], dt.float8e4)
            ones_sb = cpool.tile([P, 2, P], dt.float8e4)
            scale_sb = cpool.tile([P, 2 * H], dt.float32)
            bias_sb = cpool.tile([P, NT], dt.float32)

            # x DMAs are issued inside the h==0 prologue below (after the
            # first two weight tiles) so the PE can start chunk-paced.
            NXSPLIT = 4
            nc.vector.memset(ones_sb[:], 1.0)

            def mm_evict(dst_ap, ps0b, ps1b):
                """walrus only allows one PSUM input per vector op.  The hi*hi
                psum (ps0) stops accumulating at the end of the FIRST pass, so
                evict it via Act as soon as it stops (frees its bank early and
                keeps the slot-reuse chain off the critical path); the DVE then
                combines the correction psum: dst = ps1/16 + p0s."""
                p0s = scrpool.tile([P, C], dt.bfloat16, tag="p1s", name="p0s")
                nc.scalar.activation(p0s[:], ps0b[:], AF.Copy)
                nc.vector.scalar_tensor_tensor(
                    out=dst_ap,
                    in0=ps1b[:],
                    scalar=1.0 / RSCALE,
                    in1=p0s[:],
                    op0=ALU.mult,
                    op1=ALU.add,
                )

            def mm_passes(wh_sb, wl_sb, ps0, ps1, srch_sb, srcl_sb, kp_order=None):
                """Issue the three DoubleRow passes for one output tile:
                P0 += hi_x@hi_w;  P1 += lo_x@hi_w + hi_x@lo_w (both x16)."""
                if kp_order is None:
                    kp_order = list(range(NP))
                for i, kp in enumerate(kp_order):
                    ksl = slice(2 * kp, 2 * kp + 2)
                    for b in range(BPC):
                        nc.tensor.matmul(
                            ps0[b][:],
                            wh_sb[:, ksl, :],
                            srch_sb[:, ksl, b, :],
                            start=(i == 0),
                            stop=(i == NP - 1),
                            perf_mode=PM.DoubleRow,
                        )
                for i, kp in enumerate(kp_order):
                    ksl = slice(2 * kp, 2 * kp + 2)
                    for b in range(BPC):
                        nc.tensor.matmul(
                            ps1[b][:],
                            wl_sb[:, ksl, :],
                            srch_sb[:, ksl, b, :],
                            start=(i == 0),
                            stop=False,
                            perf_mode=PM.DoubleRow,
                        )
                for i, kp in enumerate(kp_order):
                    ksl = slice(2 * kp, 2 * kp + 2)
                    for b in range(BPC):
                        nc.tensor.matmul(
                            ps1[b][:],
                            wh_sb[:, ksl, :],
                            srcl_sb[:, ksl, b, :],
                            start=False,
                            stop=(i == NP - 1),
                            perf_mode=PM.DoubleRow,
                        )

            def alloc_ps(pool=None, tag="mm"):
                pool = pool or mmpsum
                ps0 = [
                    pool.tile([P, C], dt.float32, tag=tag, name="ps0")
                    for _ in range(BPC)
                ]
                ps1 = [
                    pool.tile([P, C], dt.float32, tag=tag, name="ps1")
                    for _ in range(BPC)
                ]
                return ps0, ps1

            def mm_tile(wh_src, wl_src, widx, dst, dst_sub, srch_sb, srcl_sb,
                        pspool=None):
                wh_sb = wpool.tile([P, NT, P], dt.float8e4, tag="w", name="wh_sb")
                nc.sync.dma_start(wh_sb[:], wh_src[widx])
                wl_sb = wpool.tile([P, NT, P], dt.float8e4, tag="w", name="wl_sb")
                nc.sync.dma_start(wl_sb[:], wl_src[widx])
                ps0, ps1 = alloc_ps(pspool, "mm" if pspool is None else "ssq")
                mm_passes(wh_sb, wl_sb, ps0, ps1, srch_sb, srcl_sb)
                for b in range(BPC):
                    mm_evict(dst[:, dst_sub, b, :], ps0[b], ps1[b])

            def issue_sq(q_sb, k_sb):
                """fp8 squares of q/k on the Act engine — one big op each.
                Issued right after the v1 tile so the Act engine computes them
                while the PE runs the v2/v3 tile groups."""
                sqs = {}
                for b in range(BPC):
                    for which, src in (("q", q_sb), ("k", k_sb)):
                        sq = sqpool.tile([P, MSUB, C], dt.float8e4, tag="sq", name="sq")
                        nc.scalar.activation(
                            sq[:], src[:, :, b, :], AF.Square, scale=1.0 / WSCALE
                        )
                        sqs[(b, which)] = sq
                return sqs

            def issue_ssq(h, sqs):
                """ssq ones-matmuls (PE DoubleRow) + rnorm = 1/||true||.
                rn_k additionally absorbs the folded per-head scale via the
                Sqrt input scale: sqrt(issq*s^2) = s/||k||."""
                rnorms = {}
                for b in range(BPC):
                    for which in ("q", "k"):
                        sq = sqs[(b, which)]
                        ssq_ps = ssqpsum.tile([P, C], dt.float32, tag="ssq", name="ssq_ps")
                        for sp in range(MSUB // 2):
                            nc.tensor.matmul(
                                ssq_ps[:],
                                ones_sb[:],
                                sq[:, 2 * sp : 2 * sp + 2, :],
                                start=(sp == 0),
                                stop=(sp == MSUB // 2 - 1),
                                perf_mode=PM.DoubleRow,
                            )
                        issq = nrmpool.tile([P, C], dt.float32, tag="issq", name="issq", bufs=1)
                        nc.vector.reciprocal(issq[:], ssq_ps[:])
                        rn = nrmpool.tile(
                            [P, C], dt.float32, tag=f"rn{which}{b}", name="rn", bufs=1
                        )
                        sc = scale_sb[:, H + h : H + h + 1] if which == "k" else 1.0
                        nc.scalar.activation(rn[:], issq[:], AF.Sqrt, scale=sc)
                        rnorms[(b, which)] = rn
                return rnorms

            def issue_kt_gelu(h, b, q_sb, k_sb, v_sb, rnorms):
                """DVE/Act-only: kernel-trick reduction + gelu + fp8 o pair.
                kt_sb[:, sub] is final right after its reduce (scale folded
                into rn_k), so each sub's gelu chain starts immediately."""
                kt_sb = nrmpool.tile([P, MSUB], dt.float32, tag="kt", name="kt_sb")
                for sub in range(MSUB):
                    vrk = scrpool.tile([P, C], dt.float32, tag="vrk", name="vrk", bufs=2)
                    nc.vector.tensor_mul(vrk[:], v_sb[:, sub, b, :], rnorms[(b, "k")][:])
                    prod = scrpool.tile([P, C], dt.float32, tag="prod", name="prod", bufs=2)
                    nc.vector.tensor_mul(prod[:], vrk[:], k_sb[:, sub, b, :])
                    nc.vector.tensor_reduce(
                        kt_sb[:, sub : sub + 1],
                        prod[:],
                        axis=mybir.AxisListType.X,
                        op=ALU.add,
                    )

                # o = gelu(q_hat * kt), stored as fp8 pair at x64 scale
                for sub in range(MSUB):
                    gin = scrpool.tile([P, C], dt.float32, tag="gin", name="gin", bufs=2)
                    nc.vector.scalar_tensor_tensor(
                        out=gin[:],
                        in0=q_sb[:, sub, b, :],
                        scalar=kt_sb[:, sub : sub + 1],
                        in1=rnorms[(b, "q")][:],
                        op0=ALU.mult,
                        op1=ALU.mult,
                    )
                    t_sb = scrpool.tile([P, C], dt.bfloat16, tag="t", name="t_sb")
                    nc.scalar.activation(t_sb[:], gin[:], act_fn)
                    osl = (slice(None), h * MSUB + sub, b, slice(None))
                    nc.scalar.activation(
                        oh_sb[osl], t_sb[:], AF.Copy, scale=OSCALE
                    )
                    d_sb = scrpool.tile([P, C], dt.bfloat16, tag="d", name="d_sb")
                    nc.vector.scalar_tensor_tensor(
                        out=d_sb[:],
                        in0=oh_sb[osl],
                        scalar=1.0 / OSCALE,
                        in1=t_sb[:],
                        op0=ALU.mult,
                        op1=ALU.subtract,
                    )
                    nc.vector.tensor_scalar_mul(
                        ol_sb[osl], d_sb[:], -OSCALE * RSCALE
                    )

            # ---------------- stage 1: q/kv + attention + gelu ----------------
            for h in range(H):
                q_sb = qkvpool.tile([P, MSUB, BPC, C], dt.bfloat16, tag="q", name="q_sb")
                k_sb = qkvpool.tile([P, MSUB, BPC, C], dt.bfloat16, tag="k", name="k_sb")
                v_sb = qkvpool.tile([P, MSUB, BPC, C], dt.bfloat16, tag="v", name="v_sb")

                if h == 0:
                    # Prologue: start the PE as soon as w-tile 0/1 hi + the
                    # first x_hi chunk land.  Open the hi*hi groups of the
                    # first two q tiles (4 psums), then the correction group
                    # of tile 0 (2 more = all 6) — chunk-paced by the x DMAs.
                    w_ = NT // NXSPLIT

                    def xdma(dst, dsrc, xs):
                        nc.sync.dma_start(
                            dst[:, xs * w_ : (xs + 1) * w_],
                            dsrc[:, xs * w_ : (xs + 1) * w_],
                        )

                    whs, wls = [], []
                    xdma(xh_sb, xh_d, 0)
                    for t in range(2):
                        wh = wpool.tile([P, NT, P], dt.float8e4, tag="w", name="wh_sb")
                        nc.sync.dma_start(wh[:], wqh_d[t])
                        whs.append(wh)
                        wl = wpool.tile([P, NT, P], dt.float8e4, tag="w", name="wl_sb")
                        nc.sync.dma_start(wl[:], wql_d[t])
                        wls.append(wl)
                        xdma(xh_sb, xh_d, t + 1)
                    xdma(xh_sb, xh_d, 3)
                    for xs in range(NXSPLIT):
                        xdma(xl_sb, xl_d, xs)
                    nc.sync.dma_start(scale_sb[:], scale_d[:])
                    nc.sync.dma_start(bias_sb[:], bias_d[:])
                    # 4 of 6 psum bufs for the two hi*hi groups; the t0
                    # correction group takes the last 2, t1's waits for t0's
                    # eviction to free slots (no deadlock).
                    ps0s = [
                        [
                            mmpsum.tile([P, C], dt.float32, tag="mm", name="ps0")
                            for _ in range(BPC)
                        ]
                        for _ in range(2)
                    ]
                    # interleave the two hi*hi groups at x-chunk granularity
                    CHUNK = NP // NXSPLIT
                    for xs in range(NXSPLIT):
                        for t in range(2):
                            for kp in range(xs * CHUNK, (xs + 1) * CHUNK):
                                ksl = slice(2 * kp, 2 * kp + 2)
                                for b in range(BPC):
                                    nc.tensor.matmul(
                                        ps0s[t][b][:],
                                        whs[t][:, ksl, :],
                                        xh_sb[:, ksl, b, :],
                                        start=(kp == 0),
                                        stop=(kp == NP - 1),
                                        perf_mode=PM.DoubleRow,
                                    )
                    ps1s = [
                        [
                            pool.tile([P, C], dt.float32, tag=tg, name="ps1")
                            for _ in range(BPC)
                        ]
                        for pool, tg in ((mmpsum, "mm"), (ssqpsum, "ssq"))
                    ]
                    # wl cross-pass for both tiles (dense work while xl loads)
                    for t in range(2):
                        for i, kp in enumerate(range(NP)):
                            ksl = slice(2 * kp, 2 * kp + 2)
                            for b in range(BPC):
                                nc.tensor.matmul(
                                    ps1s[t][b][:],
                                    wls[t][:, ksl, :],
                                    xh_sb[:, ksl, b, :],
                                    start=(i == 0),
                                    stop=False,
                                    perf_mode=PM.DoubleRow,
                                )
                    # xl cross-pass, both tiles interleaved at chunk pace
                    for xs in range(NXSPLIT):
                        for t in range(2):
                            for kp in range(xs * CHUNK, (xs + 1) * CHUNK):
                                ksl = slice(2 * kp, 2 * kp + 2)
                                for b in range(BPC):
                                    nc.tensor.matmul(
                                        ps1s[t][b][:],
                                        whs[t][:, ksl, :],
                                        xl_sb[:, ksl, b, :],
                                        start=False,
                                        stop=(kp == NP - 1),
                                        perf_mode=PM.DoubleRow,
                                    )
                    for t in range(2):
                        for b in range(BPC):
                            mm_evict(q_sb[:, t, b, :], ps0s[t][b], ps1s[t][b])
                    first_q = 2
                else:
                    first_q = 0

                for sub in range(first_q, MSUB):
                    mm_tile(wqh_d, wql_d, h * MSUB + sub, q_sb, sub, xh_sb, xl_sb)
                for sub in range(MSUB):
                    mm_tile(wkvh_d, wkvl_d, h * MSUB + sub, k_sb, sub, xh_sb, xl_sb)
                # sq on Act during the v0/v1 tile groups; ssq matmuls issued
                # after v1 so the PE reaches them with sq long since done.
                sqs = issue_sq(q_sb, k_sb)
                for sub in range(2):
                    mm_tile(wkvh_d, wkvl_d, NT + h * MSUB + sub, v_sb, sub, xh_sb, xl_sb)
                rnorms = issue_ssq(h, sqs)
                for sub in range(2, MSUB):
                    mm_tile(wkvh_d, wkvl_d, NT + h * MSUB + sub, v_sb, sub, xh_sb, xl_sb)
                for b in range(BPC):
                    issue_kt_gelu(h, b, q_sb, k_sb, v_sb, rnorms)

            # ---------------- stage 2: output projection ----------------
            # kp pairs 14/15 (= head 7's rows) are issued after all three
            # passes' earlier pairs, so the first tiles don't stall on the
            # last head's gelu outputs still in flight.
            for mt in range(NT):
                wh_sb = wpool.tile([P, NT, P], dt.float8e4, tag="w", name="wh_sb")
                nc.sync.dma_start(wh_sb[:], wph_d[mt])
                wl_sb = wpool.tile([P, NT, P], dt.float8e4, tag="w", name="wl_sb")
                nc.sync.dma_start(wl_sb[:], wpl_d[mt])
                ps0, ps1 = alloc_ps()
                for kps in (range(NP - 2), range(NP - 2, NP)):
                    for kp in kps:
                        ksl = slice(2 * kp, 2 * kp + 2)
                        for b in range(BPC):
                            nc.tensor.matmul(
                                ps0[b][:],
                                wh_sb[:, ksl, :],
                                oh_sb[:, ksl, b, :],
                                start=(kp == 0),
                                stop=(kp == NP - 1),
                                perf_mode=PM.DoubleRow,
                            )
                    for kp in kps:
                        ksl = slice(2 * kp, 2 * kp + 2)
                        for b in range(BPC):
                            nc.tensor.matmul(
                                ps1[b][:],
                                wh_sb[:, ksl, :],
                                ol_sb[:, ksl, b, :],
                                start=(kp == 0),
                                stop=False,
                                perf_mode=PM.DoubleRow,
                            )
                    for kp in kps:
                        ksl = slice(2 * kp, 2 * kp + 2)
                        for b in range(BPC):
                            nc.tensor.matmul(
                                ps1[b][:],
                                wl_sb[:, ksl, :],
                                oh_sb[:, ksl, b, :],
                                start=False,
                                stop=(kp == NP - 1),
                                perf_mode=PM.DoubleRow,
                            )
                for b in range(BPC):
                    # p0s = ps0/4096 + bias (early: ps0 stops after pass 1);
                    # ob = ps1/(16*4096) + p0s
                    p0s = outpool.tile([P, C], dt.float32, tag="p1s2", name="p0s")
                    nc.scalar.activation(
                        p0s[:], ps0[b][:], AF.Identity,
                        scale=1.0 / (WSCALE * OSCALE),
                        bias=bias_sb[:, mt : mt + 1],
                    )
                    ob = outpool.tile([P, C], dt.float32, tag="ob", name="ob")
                    nc.vector.scalar_tensor_tensor(
                        out=ob[:],
                        in0=ps1[b][:],
                        scalar=1.0 / (RSCALE * WSCALE * OSCALE),
                        in1=p0s[:],
                        op0=ALU.mult,
                        op1=ALU.add,
                    )
                    nc.sync.dma_start(out_d[b, mt], ob[:])

    nc.compile()
    return nc


def _split_fp8(a):
    """a (fp32, ~N(0,1) scale) -> (hi, lo) fp8 with a ~= hi + lo/16."""
    hi = a.astype(F8)
    lo = ((a - hi.astype(np.float32)) * RSCALE).astype(F8)
    return hi, lo


def _prep_inputs(x, wq, wkv, wproj, bproj, scale):
    x = np.asarray(x, dtype=np.float32)
    wq = np.asarray(wq, dtype=np.float32)
    wkv = np.asarray(wkv, dtype=np.float32)
    wproj = np.asarray(wproj, dtype=np.float32)
    bproj = np.asarray(bproj, dtype=np.float32)
    scale = np.asarray(scale, dtype=np.float32)

    # W[mt, p, nt, j] = w[mt*128 + j, nt*128 + p]  (transposed tile layout),
    # scaled x64 then split into fp8 hi/lo pairs.
    def wtiles(w, mtiles):
        wt = np.ascontiguousarray(
            w.reshape(mtiles, P, NT, P).transpose(0, 3, 2, 1)
        ) * WSCALE
        return _split_fp8(wt)

    Wqh, Wql = wtiles(wq, NT)
    Wkvh, Wkvl = wtiles(wkv, 2 * NT)
    Wph, Wpl = wtiles(wproj, NT)

    # X[core][p, nt, b, c] = x[2*core + b, nt*128 + p, c]
    Xall = np.ascontiguousarray(
        x.reshape(NCORES, BPC, NT, P, C).transpose(0, 3, 2, 1, 4)
    )
    Xh, Xl = _split_fp8(Xall)

    # stage-2 eviction: p1s2 = ps1/(16*4096) + bias; ob = ps0/4096 + p1s2
    bias = np.ascontiguousarray(bproj.reshape(NT, P).T)
    # kt folding: kt_raw = 4096 * sum(khat*v); want kt = kt_true*scale/64.
    # The factor scale/64^3 is folded into rn_k as sqrt(issq * s^2), so
    # column H+h carries s^2.
    s1 = scale.reshape(1, H) / (WSCALE * WSCALE * WSCALE)
    scale_b = np.ascontiguousarray(
        np.broadcast_to(np.concatenate([s1, s1 * s1], axis=1), (P, 2 * H))
    ).astype(np.float32)

    in_maps = []
    for c in range(NCORES):
        in_maps.append(
            {
                "x_hi": Xh[c],
                "x_lo": Xl[c],
                "wq_hi": Wqh,
                "wq_lo": Wql,
                "wkv_hi": Wkvh,
                "wkv_lo": Wkvl,
                "wproj_hi": Wph,
                "wproj_lo": Wpl,
                "bias": bias,
                "scale": scale_b,
            }
        )
    return in_maps


def kernel(x, wq, wkv, wproj, bproj, scale):
    global _NC, _LAST_RESULTS
    if _NC is None:
        _NC = _build_nc()

    in_maps = _prep_inputs(x, wq, wkv, wproj, bproj, scale)
    res = run_bass_kernel_spmd(_NC, in_maps, core_ids=list(range(NCORES)))
    _LAST_RESULTS = res

    outs = [res.results[c]["out"].reshape(BPC, N, C) for c in range(NCORES)]
    return np.ascontiguousarray(np.concatenate(outs, axis=0), dtype=np.float32)


# revision 33
# speedup vs baseline: 1.0360x; 1.0360x over previous
"""Trainium2 Bass kernel: dimensional (channel) attention token-mixing block.

Computation (per batch b):
    xt = x[b].T                               # [C, N]
    q  = xt @ wq.T                            # [C, N]   (heads: N = H*NH)
    kv = xt @ wkv.T                           # [C, 2N]
    q, k normalized over NH per (c, head)
    kt[h] = sum_c k_hat[c,h,:] * v[c,h,:] * scale[h]     # [NH]
    o  = gelu(q_hat * kt)                     # [C, N]
    out[b] = (o @ wproj.T + bproj).T          # [N, C]

Sharding: data-parallel over B across 8 cores (2 batches/core), weights
replicated.  All tensors kept in "transposed" [n, c] layout on device so the
contraction dim n always lies on SBUF partitions.

Precision strategy: all big matmuls run as fp8(e4m3) DoubleRow pairs with
residual compensation.  Each operand A is split A = A_hi + A_lo/16 with
A_hi = fp8(A), A_lo = fp8(16*(A - A_hi)).  Then
    A@W = [A_hi@W_hi] + ([A_lo@W_hi] + [A_hi@W_lo]) / 16   (lo*lo dropped)
The hi*hi product accumulates in one PSUM bank (P0), both cross terms in a
second (P1); the eviction to SBUF computes P0 + P1/16 in a single
scalar_tensor_tensor op.  DoubleRow contracts 256 rows per instruction at
0.5 cycles/output-row, so the 3-term compensated matmul runs 1.33x faster
than bf16 with slightly BETTER accuracy (residuals capture bf16-level
precision).

Scale folding (weights stored x64 so fp8 sees ~N(0,1) values):
  - q_sb/k_sb = 64*true: killed by the q/k normalization (rnorm computed
    from ssq*2^-12 gives exactly 1/||true||).
  - v_sb = 64*true and k_sb*rnorm = 64*khat: the host folds 1/(64*4096)
    into the per-head `scale` tensor.
  - o is stored as fp8 pair at 64*true (o_hi = fp8(64*gelu),
    o_lo = fp8(1024*residual)); wproj x64: the host folds the 2^-12
    descale + bias into the final tensor_scalar (bias pre-scaled x4096).
"""

import sys

import numpy as np
import ml_dtypes

if "/opt/trn_rl_repo" not in sys.path:
    sys.path.insert(0, "/opt/trn_rl_repo")

import concourse.bass as bass
import concourse.bacc as bacc
import concourse.mybir as mybir
import concourse.tile as tile
from concourse.bass_utils import run_bass_kernel_spmd

B, N, C, H = 16, 4096, 384, 8
NH = N // H          # 512
P = 128
NT = N // P          # 32 contraction chunks of 128
NP = NT // 2         # 16 DoubleRow k-tile pairs
MSUB = NH // P       # 4 output row-subtiles per head
NCORES = 8
BPC = B // NCORES    # batches per core

dt = mybir.dt
AF = mybir.ActivationFunctionType
ALU = mybir.AluOpType
PM = mybir.MatmulPerfMode
F8 = ml_dtypes.float8_e4m3
BF16 = ml_dtypes.bfloat16

WSCALE = 64.0        # weights stored x64 (fp8 normal range)
RSCALE = 16.0        # residual (lo) tensors stored x16
OSCALE = 64.0        # gelu output stored x64

_NC = None
_LAST_RESULTS = None


def _build_nc(act_fn=None):
    if act_fn is None:
        act_fn = AF.Gelu
    nc = bacc.Bacc("TRN2", target_bir_lowering=False, debug=False)

    xh_d = nc.dram_tensor("x_hi", [P, NT, BPC, C], dt.float8e4, kind="ExternalInput")
    xl_d = nc.dram_tensor("x_lo", [P, NT, BPC, C], dt.float8e4, kind="ExternalInput")
    wqh_d = nc.dram_tensor("wq_hi", [NT, P, NT, P], dt.float8e4, kind="ExternalInput")
    wql_d = nc.dram_tensor("wq_lo", [NT, P, NT, P], dt.float8e4, kind="ExternalInput")
    wkvh_d = nc.dram_tensor("wkv_hi", [2 * NT, P, NT, P], dt.float8e4, kind="ExternalInput")
    wkvl_d = nc.dram_tensor("wkv_lo", [2 * NT, P, NT, P], dt.float8e4, kind="ExternalInput")
    wph_d = nc.dram_tensor("wproj_hi", [NT, P, NT, P], dt.float8e4, kind="ExternalInput")
    wpl_d = nc.dram_tensor("wproj_lo", [NT, P, NT, P], dt.float8e4, kind="ExternalInput")
    bias_d = nc.dram_tensor("bias", [P, NT], dt.float32, kind="ExternalInput")
    # col h: unused; col H+h: (scale[h] * 2^-18)^2, folded into rn_k via the
    # Sqrt activation's input scale.
    scale_d = nc.dram_tensor("scale", [P, 2 * H], dt.float32, kind="ExternalInput")
    out_d = nc.dram_tensor("out", [BPC, NT, P, C], dt.float32, kind="ExternalOutput")

    with tile.TileContext(nc) as tc:
        with (
            tc.tile_pool(name="const", bufs=1) as cpool,
            tc.tile_pool(name="wts", bufs=8) as wpool,
            tc.tile_pool(name="qkv", bufs=1) as qkvpool,
            tc.tile_pool(name="sqp", bufs=4) as sqpool,
            tc.tile_pool(name="nrm", bufs=2) as nrmpool,
            tc.tile_pool(name="scr", bufs=4) as scrpool,
            tc.tile_pool(name="outp", bufs=4) as outpool,
            tc.tile_pool(name="mmps", bufs=6, space="PSUM") as mmpsum,
            tc.tile_pool(name="ssqps", bufs=2, space="PSUM") as ssqpsum,
        ):
            xh_sb = cpool.tile([P, NT, BPC, C], dt.float8e4)
            xl_sb = cpool.tile([P, NT, BPC, C], dt.float8e4)
            oh_sb = cpool.tile([P, NT, BPC, C], dt.float8e4)
            ol_sb = cpool.tile([P, NT, BPC, C], dt.float8e4)
            ones_sb = cpool.tile([P, 2, P], dt.float8e4)
            scale_sb = cpool.tile([P, 2 * H], dt.float32)
            bias_sb = cpool.tile([P, NT], dt.float32)

            # x DMAs are issued inside the h==0 prologue below (after the
            # first two weight tiles) so the PE can start chunk-paced.
            NXSPLIT = 4
            nc.vector.memset(ones_sb[:], 1.0)

            def mm_evict(dst_ap, ps0b, ps1b):
                """walrus only allows one PSUM input per vector op.  The hi*hi
                psum (ps0) stops accumulating at the end of the FIRST pass, so
                evict it via Act as soon as it stops (frees its bank early and
                keeps the slot-reuse chain off the critical path); the DVE then
                combines the correction psum: dst = ps1/16 + p0s."""
                p0s = scrpool.tile([P, C], dt.bfloat16, tag="p1s", name="p0s")
                nc.scalar.activation(p0s[:], ps0b[:], AF.Copy)
                nc.vector.scalar_tensor_tensor(
                    out=dst_ap,
                    in0=ps1b[:],
                    scalar=1.0 / RSCALE,
                    in1=p0s[:],
                    op0=ALU.mult,
                    op1=ALU.add,
                )

            # kp pairs whose correction (cross) passes are skipped: spends a
            # measured slice of the error budget (err ~= 3.0e-2*sqrt(n/16/4
            # summed over groups)) to cut PE time by ~2*80ns*2b per tile per
            # dropped pair.  kp 0 and NP-1 must never be dropped (start/stop
            # flags live there).
            DROP = frozenset({7})

            def mm_passes(wh_sb, wl_sb, ps0, ps1, srch_sb, srcl_sb, kp_order=None,
                          drop=DROP):
                """Issue the three DoubleRow passes for one output tile:
                P0 += hi_x@hi_w;  P1 += lo_x@hi_w + hi_x@lo_w (both x16)."""
                if kp_order is None:
                    kp_order = list(range(NP))
                for i, kp in enumerate(kp_order):
                    ksl = slice(2 * kp, 2 * kp + 2)
                    for b in range(BPC):
                        nc.tensor.matmul(
                            ps0[b][:],
                            wh_sb[:, ksl, :],
                            srch_sb[:, ksl, b, :],
                            start=(i == 0),
                            stop=(i == NP - 1),
                            perf_mode=PM.DoubleRow,
                        )
                for i, kp in enumerate(kp_order):
                    if kp in drop:
                        continue
                    ksl = slice(2 * kp, 2 * kp + 2)
                    for b in range(BPC):
                        nc.tensor.matmul(
                            ps1[b][:],
                            wl_sb[:, ksl, :],
                            srch_sb[:, ksl, b, :],
                            start=(i == 0),
                            stop=False,
                            perf_mode=PM.DoubleRow,
                        )
                for i, kp in enumerate(kp_order):
                    if kp in drop:
                        continue
                    ksl = slice(2 * kp, 2 * kp + 2)
                    for b in range(BPC):
                        nc.tensor.matmul(
                            ps1[b][:],
                            wh_sb[:, ksl, :],
                            srcl_sb[:, ksl, b, :],
                            start=False,
                            stop=(i == NP - 1),
                            perf_mode=PM.DoubleRow,
                        )

            def alloc_ps(pool=None, tag="mm"):
                pool = pool or mmpsum
                ps0 = [
                    pool.tile([P, C], dt.float32, tag=tag, name="ps0")
                    for _ in range(BPC)
                ]
                ps1 = [
                    pool.tile([P, C], dt.float32, tag=tag, name="ps1")
                    for _ in range(BPC)
                ]
                return ps0, ps1

            def mm_tile(wh_src, wl_src, widx, dst, dst_sub, srch_sb, srcl_sb,
                        pspool=None):
                wh_sb = wpool.tile([P, NT, P], dt.float8e4, tag="w", name="wh_sb")
                nc.sync.dma_start(wh_sb[:], wh_src[widx])
                wl_sb = wpool.tile([P, NT, P], dt.float8e4, tag="w", name="wl_sb")
                nc.sync.dma_start(wl_sb[:], wl_src[widx])
                ps0, ps1 = alloc_ps(pspool, "mm" if pspool is None else "ssq")
                mm_passes(wh_sb, wl_sb, ps0, ps1, srch_sb, srcl_sb)
                for b in range(BPC):
                    mm_evict(dst[:, dst_sub, b, :], ps0[b], ps1[b])

            def issue_sq(q_sb, k_sb):
                """fp8 squares of q/k on the Act engine — one big op each.
                Issued right after the v1 tile so the Act engine computes them
                while the PE runs the v2/v3 tile groups."""
                sqs = {}
                for b in range(BPC):
                    for which, src in (("q", q_sb), ("k", k_sb)):
                        sq = sqpool.tile([P, MSUB, C], dt.float8e4, tag="sq", name="sq")
                        nc.scalar.activation(
                            sq[:], src[:, :, b, :], AF.Square, scale=1.0 / WSCALE
                        )
                        sqs[(b, which)] = sq
                return sqs

            def issue_ssq(h, sqs):
                """ssq ones-matmuls (PE DoubleRow) + rnorm = 1/||true||.
                rn_k additionally absorbs the folded per-head scale via the
                Sqrt input scale: sqrt(issq*s^2) = s/||k||."""
                rnorms = {}
                for b in range(BPC):
                    for which in ("q", "k"):
                        sq = sqs[(b, which)]
                        ssq_ps = ssqpsum.tile([P, C], dt.float32, tag="ssq", name="ssq_ps")
                        for sp in range(MSUB // 2):
                            nc.tensor.matmul(
                                ssq_ps[:],
                                ones_sb[:],
                                sq[:, 2 * sp : 2 * sp + 2, :],
                                start=(sp == 0),
                                stop=(sp == MSUB // 2 - 1),
                                perf_mode=PM.DoubleRow,
                            )
                        issq = nrmpool.tile([P, C], dt.float32, tag="issq", name="issq", bufs=1)
                        nc.vector.reciprocal(issq[:], ssq_ps[:])
                        rn = nrmpool.tile(
                            [P, C], dt.float32, tag=f"rn{which}{b}", name="rn", bufs=1
                        )
                        sc = scale_sb[:, H + h : H + h + 1] if which == "k" else 1.0
                        nc.scalar.activation(rn[:], issq[:], AF.Sqrt, scale=sc)
                        rnorms[(b, which)] = rn
                return rnorms

            def issue_kt(h, b, k_sb, v_sb, rnorms):
                """kt_sb[:, sub] is final right after its reduce (scale
                folded into rn_k)."""
                kt_sb = nrmpool.tile([P, MSUB], dt.float32, tag="kt", name="kt_sb")
                for sub in range(MSUB):
                    vrk = scrpool.tile([P, C], dt.float32, tag="vrk", name="vrk", bufs=2)
                    nc.vector.tensor_mul(vrk[:], v_sb[:, sub, b, :], rnorms[(b, "k")][:])
                    prod = scrpool.tile([P, C], dt.float32, tag="prod", name="prod", bufs=2)
                    nc.vector.tensor_mul(prod[:], vrk[:], k_sb[:, sub, b, :])
                    nc.vector.tensor_reduce(
                        kt_sb[:, sub : sub + 1],
                        prod[:],
                        axis=mybir.AxisListType.X,
                        op=ALU.add,
                    )
                return kt_sb

            def issue_gelu(h, b, sub, q_sb, kt_sb, rnorms):
                """o = gelu(q_hat * kt), stored as fp8 pair at x64 scale."""
                gin = scrpool.tile([P, C], dt.float32, tag="gin", name="gin", bufs=2)
                nc.vector.scalar_tensor_tensor(
                    out=gin[:],
                    in0=q_sb[:, sub, b, :],
                    scalar=kt_sb[:, sub : sub + 1],
                    in1=rnorms[(b, "q")][:],
                    op0=ALU.mult,
                    op1=ALU.mult,
                )
                t_sb = scrpool.tile([P, C], dt.bfloat16, tag="t", name="t_sb")
                nc.scalar.activation(t_sb[:], gin[:], act_fn)
                osl = (slice(None), h * MSUB + sub, b, slice(None))
                nc.scalar.activation(
                    oh_sb[osl], t_sb[:], AF.Copy, scale=OSCALE
                )
                d_sb = scrpool.tile([P, C], dt.bfloat16, tag="d", name="d_sb")
                nc.vector.scalar_tensor_tensor(
                    out=d_sb[:],
                    in0=oh_sb[osl],
                    scalar=1.0 / OSCALE,
                    in1=t_sb[:],
                    op0=ALU.mult,
                    op1=ALU.subtract,
                )
                nc.vector.tensor_scalar_mul(
                    ol_sb[osl], d_sb[:], -OSCALE * RSCALE
                )

            def issue_kt_gelu(h, b, q_sb, k_sb, v_sb, rnorms):
                kt_sb = issue_kt(h, b, k_sb, v_sb, rnorms)
                for sub in range(MSUB):
                    issue_gelu(h, b, sub, q_sb, kt_sb, rnorms)

            # ---------------- stage 1: q/kv + attention + gelu ----------------
            for h in range(H):
                q_sb = qkvpool.tile([P, MSUB, BPC, C], dt.bfloat16, tag="q", name="q_sb")
                k_sb = qkvpool.tile([P, MSUB, BPC, C], dt.bfloat16, tag="k", name="k_sb")
                v_sb = qkvpool.tile([P, MSUB, BPC, C], dt.bfloat16, tag="v", name="v_sb")

                if h == 0:
                    # Prologue: start the PE as soon as w-tile 0/1 hi + the
                    # first x_hi chunk land.  Open the hi*hi groups of the
                    # first two q tiles (4 psums), then the correction group
                    # of tile 0 (2 more = all 6) — chunk-paced by the x DMAs.
                    w_ = NT // NXSPLIT

                    def xdma(dst, dsrc, xs):
                        nc.sync.dma_start(
                            dst[:, xs * w_ : (xs + 1) * w_],
                            dsrc[:, xs * w_ : (xs + 1) * w_],
                        )

                    whs, wls = [], []
                    xdma(xh_sb, xh_d, 0)
                    for t in range(2):
                        wh = wpool.tile([P, NT, P], dt.float8e4, tag="w", name="wh_sb")
                        nc.sync.dma_start(wh[:], wqh_d[t])
                        whs.append(wh)
                        wl = wpool.tile([P, NT, P], dt.float8e4, tag="w", name="wl_sb")
                        nc.sync.dma_start(wl[:], wql_d[t])
                        wls.append(wl)
                        xdma(xh_sb, xh_d, t + 1)
                    xdma(xh_sb, xh_d, 3)
                    for xs in range(NXSPLIT):
                        xdma(xl_sb, xl_d, xs)
                    nc.sync.dma_start(scale_sb[:], scale_d[:])
                    nc.sync.dma_start(bias_sb[:], bias_d[:])
                    # 4 of 6 psum bufs for the two hi*hi groups; the t0
                    # correction group takes the last 2, t1's waits for t0's
                    # eviction to free slots (no deadlock).
                    ps0s = [
                        [
                            mmpsum.tile([P, C], dt.float32, tag="mm", name="ps0")
                            for _ in range(BPC)
                        ]
                        for _ in range(2)
                    ]
                    # interleave the two hi*hi groups at x-chunk granularity
                    CHUNK = NP // NXSPLIT
                    for xs in range(NXSPLIT):
                        for t in range(2):
                            for kp in range(xs * CHUNK, (xs + 1) * CHUNK):
                                ksl = slice(2 * kp, 2 * kp + 2)
                                for b in range(BPC):
                                    nc.tensor.matmul(
                                        ps0s[t][b][:],
                                        whs[t][:, ksl, :],
                                        xh_sb[:, ksl, b, :],
                                        start=(kp == 0),
                                        stop=(kp == NP - 1),
                                        perf_mode=PM.DoubleRow,
                                    )
                    ps1s = [
                        [
                            pool.tile([P, C], dt.float32, tag=tg, name="ps1")
                            for _ in range(BPC)
                        ]
                        for pool, tg in ((mmpsum, "mm"), (ssqpsum, "ssq"))
                    ]
                    # wl cross-pass for both tiles (dense work while xl loads)
                    for t in range(2):
                        for i, kp in enumerate(range(NP)):
                            ksl = slice(2 * kp, 2 * kp + 2)
                            for b in range(BPC):
                                nc.tensor.matmul(
                                    ps1s[t][b][:],
                                    wls[t][:, ksl, :],
                                    xh_sb[:, ksl, b, :],
                                    start=(i == 0),
                                    stop=False,
                                    perf_mode=PM.DoubleRow,
                                )
                    # xl cross-pass, both tiles interleaved at chunk pace
                    for xs in range(NXSPLIT):
                        for t in range(2):
                            for kp in range(xs * CHUNK, (xs + 1) * CHUNK):
                                ksl = slice(2 * kp, 2 * kp + 2)
                                for b in range(BPC):
                                    nc.tensor.matmul(
                                        ps1s[t][b][:],
                                        whs[t][:, ksl, :],
                                        xl_sb[:, ksl, b, :],
                                        start=False,
                                        stop=(kp == NP - 1),
                                        perf_mode=PM.DoubleRow,
                                    )
                    for t in range(2):
                        for b in range(BPC):
                            mm_evict(q_sb[:, t, b, :], ps0s[t][b], ps1s[t][b])
                    first_q = 2
                else:
                    first_q = 0

                for sub in range(first_q, MSUB):
                    mm_tile(wqh_d, wql_d, h * MSUB + sub, q_sb, sub, xh_sb, xl_sb)
                for sub in range(MSUB):
                    mm_tile(wkvh_d, wkvl_d, h * MSUB + sub, k_sb, sub, xh_sb, xl_sb)
                # sq on Act during the v0/v1 tile groups; ssq matmuls issued
                # after v1 so the PE reaches them with sq long since done.
                sqs = issue_sq(q_sb, k_sb)
                for sub in range(2):
                    mm_tile(wkvh_d, wkvl_d, NT + h * MSUB + sub, v_sb, sub, xh_sb, xl_sb)
                rnorms = issue_ssq(h, sqs)
                for sub in range(2, MSUB):
                    mm_tile(wkvh_d, wkvl_d, NT + h * MSUB + sub, v_sb, sub, xh_sb, xl_sb)
                if h < H - 1:
                    for b in range(BPC):
                        issue_kt_gelu(h, b, q_sb, k_sb, v_sb, rnorms)
                else:
                    kts = [issue_kt(h, b, k_sb, v_sb, rnorms) for b in range(BPC)]
                    for sub in range(MSUB):
                        for b in range(BPC):
                            issue_gelu(h, b, sub, q_sb, kts[b], rnorms)

            # ---------------- stage 2: output projection ----------------
            # kp pairs 14/15 (= head 7's rows) are issued after all three
            # passes' earlier pairs, so the first tiles don't stall on the
            # last head's gelu outputs still in flight.
            for mt in range(NT):
                wh_sb = wpool.tile([P, NT, P], dt.float8e4, tag="w", name="wh_sb")
                nc.sync.dma_start(wh_sb[:], wph_d[mt])
                wl_sb = wpool.tile([P, NT, P], dt.float8e4, tag="w", name="wl_sb")
                nc.sync.dma_start(wl_sb[:], wpl_d[mt])
                ps0, ps1 = alloc_ps()
                for kps in (range(NP - 2), range(NP - 2, NP)):
                    for kp in kps:
                        ksl = slice(2 * kp, 2 * kp + 2)
                        for b in range(BPC):
                            nc.tensor.matmul(
                                ps0[b][:],
                                wh_sb[:, ksl, :],
                                oh_sb[:, ksl, b, :],
                                start=(kp == 0),
                                stop=(kp == NP - 1),
                                perf_mode=PM.DoubleRow,
                            )
                    for kp in kps:
                        if kp in DROP:
                            continue
                        ksl = slice(2 * kp, 2 * kp + 2)
                        for b in range(BPC):
                            nc.tensor.matmul(
                                ps1[b][:],
                                wh_sb[:, ksl, :],
                                ol_sb[:, ksl, b, :],
                                start=(kp == 0),
                                stop=False,
                                perf_mode=PM.DoubleRow,
                            )
                    for kp in kps:
                        if kp in DROP:
                            continue
                        ksl = slice(2 * kp, 2 * kp + 2)
                        for b in range(BPC):
                            nc.tensor.matmul(
                                ps1[b][:],
                                wl_sb[:, ksl, :],
                                oh_sb[:, ksl, b, :],
                                start=False,
                                stop=(kp == NP - 1),
                                perf_mode=PM.DoubleRow,
                            )
                for b in range(BPC):
                    # p0s = ps0/4096 + bias (early: ps0 stops after pass 1);
                    # ob = ps1/(16*4096) + p0s
                    p0s = outpool.tile([P, C], dt.float32, tag="p1s2", name="p0s")
                    nc.scalar.activation(
                        p0s[:], ps0[b][:], AF.Identity,
                        scale=1.0 / (WSCALE * OSCALE),
                        bias=bias_sb[:, mt : mt + 1],
                    )
                    ob = outpool.tile([P, C], dt.float32, tag="ob", name="ob")
                    nc.vector.scalar_tensor_tensor(
                        out=ob[:],
                        in0=ps1[b][:],
                        scalar=1.0 / (RSCALE * WSCALE * OSCALE),
                        in1=p0s[:],
                        op0=ALU.mult,
                        op1=ALU.add,
                    )
                    nc.sync.dma_start(out_d[b, mt], ob[:])

    nc.compile()
    return nc


def _split_fp8(a):
    """a (fp32, ~N(0,1) scale) -> (hi, lo) fp8 with a ~= hi + lo/16."""
    hi = a.astype(F8)
    lo = ((a - hi.astype(np.float32)) * RSCALE).astype(F8)
    return hi, lo


def _prep_inputs(x, wq, wkv, wproj, bproj, scale):
    x = np.asarray(x, dtype=np.float32)
    wq = np.asarray(wq, dtype=np.float32)
    wkv = np.asarray(wkv, dtype=np.float32)
    wproj = np.asarray(wproj, dtype=np.float32)
    bproj = np.asarray(bproj, dtype=np.float32)
    scale = np.asarray(scale, dtype=np.float32)

    # W[mt, p, nt, j] = w[mt*128 + j, nt*128 + p]  (transposed tile layout),
    # scaled x64 then split into fp8 hi/lo pairs.
    def wtiles(w, mtiles):
        wt = np.ascontiguousarray(
            w.reshape(mtiles, P, NT, P).transpose(0, 3, 2, 1)
        ) * WSCALE
        return _split_fp8(wt)

    Wqh, Wql = wtiles(wq, NT)
    Wkvh, Wkvl = wtiles(wkv, 2 * NT)
    Wph, Wpl = wtiles(wproj, NT)

    # X[core][p, nt, b, c] = x[2*core + b, nt*128 + p, c]
    Xall = np.ascontiguousarray(
        x.reshape(NCORES, BPC, NT, P, C).transpose(0, 3, 2, 1, 4)
    )
    Xh, Xl = _split_fp8(Xall)

    # stage-2 eviction: p1s2 = ps1/(16*4096) + bias; ob = ps0/4096 + p1s2
    bias = np.ascontiguousarray(bproj.reshape(NT, P).T)
    # kt folding: kt_raw = 4096 * sum(khat*v); want kt = kt_true*scale/64.
    # The factor scale/64^3 is folded into rn_k as sqrt(issq * s^2), so
    # column H+h carries s^2.
    s1 = scale.reshape(1, H) / (WSCALE * WSCALE * WSCALE)
    scale_b = np.ascontiguousarray(
        np.broadcast_to(np.concatenate([s1, s1 * s1], axis=1), (P, 2 * H))
    ).astype(np.float32)

    in_maps = []
    for c in range(NCORES):
        in_maps.append(
            {
                "x_hi": Xh[c],
                "x_lo": Xl[c],
                "wq_hi": Wqh,
                "wq_lo": Wql,
                "wkv_hi": Wkvh,
                "wkv_lo": Wkvl,
                "wproj_hi": Wph,
                "wproj_lo": Wpl,
                "bias": bias,
                "scale": scale_b,
            }
        )
    return in_maps


def kernel(x, wq, wkv, wproj, bproj, scale):
    global _NC, _LAST_RESULTS
    if _NC is None:
        _NC = _build_nc()

    in_maps = _prep_inputs(x, wq, wkv, wproj, bproj, scale)
    res = run_bass_kernel_spmd(_NC, in_maps, core_ids=list(range(NCORES)))
    _LAST_RESULTS = res

    outs = [res.results[c]["out"].reshape(BPC, N, C) for c in range(NCORES)]
    return np.ascontiguousarray(np.concatenate(outs, axis=0), dtype=np.float32)


# revision 35
# speedup vs baseline: 1.0367x; 1.0007x over previous
"""Trainium2 Bass kernel: dimensional (channel) attention token-mixing block.

Computation (per batch b):
    xt = x[b].T                               # [C, N]
    q  = xt @ wq.T                            # [C, N]   (heads: N = H*NH)
    kv = xt @ wkv.T                           # [C, 2N]
    q, k normalized over NH per (c, head)
    kt[h] = sum_c k_hat[c,h,:] * v[c,h,:] * scale[h]     # [NH]
    o  = gelu(q_hat * kt)                     # [C, N]
    out[b] = (o @ wproj.T + bproj).T          # [N, C]

Sharding: data-parallel over B across 8 cores (2 batches/core), weights
replicated.  All tensors kept in "transposed" [n, c] layout on device so the
contraction dim n always lies on SBUF partitions.

Precision strategy: all big matmuls run as fp8(e4m3) DoubleRow pairs with
residual compensation.  Each operand A is split A = A_hi + A_lo/16 with
A_hi = fp8(A), A_lo = fp8(16*(A - A_hi)).  Then
    A@W = [A_hi@W_hi] + ([A_lo@W_hi] + [A_hi@W_lo]) / 16   (lo*lo dropped)
The hi*hi product accumulates in one PSUM bank (P0), both cross terms in a
second (P1); the eviction to SBUF computes P0 + P1/16 in a single
scalar_tensor_tensor op.  DoubleRow contracts 256 rows per instruction at
0.5 cycles/output-row, so the 3-term compensated matmul runs 1.33x faster
than bf16 with slightly BETTER accuracy (residuals capture bf16-level
precision).

Scale folding (weights stored x64 so fp8 sees ~N(0,1) values):
  - q_sb/k_sb = 64*true: killed by the q/k normalization (rnorm computed
    from ssq*2^-12 gives exactly 1/||true||).
  - v_sb = 64*true and k_sb*rnorm = 64*khat: the host folds 1/(64*4096)
    into the per-head `scale` tensor.
  - o is stored as fp8 pair at 64*true (o_hi = fp8(64*gelu),
    o_lo = fp8(1024*residual)); wproj x64: the host folds the 2^-12
    descale + bias into the final tensor_scalar (bias pre-scaled x4096).
"""

import sys

import numpy as np
import ml_dtypes

if "/opt/trn_rl_repo" not in sys.path:
    sys.path.insert(0, "/opt/trn_rl_repo")

import concourse.bass as bass
import concourse.bacc as bacc
import concourse.mybir as mybir
import concourse.tile as tile
from concourse.bass_utils import run_bass_kernel_spmd

B, N, C, H = 16, 4096, 384, 8
NH = N // H          # 512
P = 128
NT = N // P          # 32 contraction chunks of 128
NP = NT // 2         # 16 DoubleRow k-tile pairs
MSUB = NH // P       # 4 output row-subtiles per head
NCORES = 8
BPC = B // NCORES    # batches per core

dt = mybir.dt
AF = mybir.ActivationFunctionType
ALU = mybir.AluOpType
PM = mybir.MatmulPerfMode
F8 = ml_dtypes.float8_e4m3
BF16 = ml_dtypes.bfloat16

WSCALE = 64.0        # weights stored x64 (fp8 normal range)
RSCALE = 16.0        # residual (lo) tensors stored x16
OSCALE = 64.0        # gelu output stored x64

_NC = None
_LAST_RESULTS = None


def _build_nc(act_fn=None):
    if act_fn is None:
        act_fn = AF.Gelu
    nc = bacc.Bacc("TRN2", target_bir_lowering=False, debug=False)

    xh_d = nc.dram_tensor("x_hi", [P, NT, BPC, C], dt.float8e4, kind="ExternalInput")
    xl_d = nc.dram_tensor("x_lo", [P, NT, BPC, C], dt.float8e4, kind="ExternalInput")
    wqh_d = nc.dram_tensor("wq_hi", [NT, P, NT, P], dt.float8e4, kind="ExternalInput")
    wql_d = nc.dram_tensor("wq_lo", [NT, P, NT, P], dt.float8e4, kind="ExternalInput")
    wkvh_d = nc.dram_tensor("wkv_hi", [2 * NT, P, NT, P], dt.float8e4, kind="ExternalInput")
    wkvl_d = nc.dram_tensor("wkv_lo", [2 * NT, P, NT, P], dt.float8e4, kind="ExternalInput")
    wph_d = nc.dram_tensor("wproj_hi", [NT, P, NT, P], dt.float8e4, kind="ExternalInput")
    wpl_d = nc.dram_tensor("wproj_lo", [NT, P, NT, P], dt.float8e4, kind="ExternalInput")
    bias_d = nc.dram_tensor("bias", [P, NT], dt.float32, kind="ExternalInput")
    # col h: unused; col H+h: (scale[h] * 2^-18)^2, folded into rn_k via the
    # Sqrt activation's input scale.
    scale_d = nc.dram_tensor("scale", [P, 2 * H], dt.float32, kind="ExternalInput")
    out_d = nc.dram_tensor("out", [BPC, NT, P, C], dt.float32, kind="ExternalOutput")

    with tile.TileContext(nc) as tc:
        with (
            tc.tile_pool(name="const", bufs=1) as cpool,
            tc.tile_pool(name="wts", bufs=8) as wpool,
            tc.tile_pool(name="qkv", bufs=1) as qkvpool,
            tc.tile_pool(name="sqp", bufs=4) as sqpool,
            tc.tile_pool(name="nrm", bufs=2) as nrmpool,
            tc.tile_pool(name="scr", bufs=4) as scrpool,
            tc.tile_pool(name="outp", bufs=4) as outpool,
            tc.tile_pool(name="mmps", bufs=6, space="PSUM") as mmpsum,
            tc.tile_pool(name="ssqps", bufs=2, space="PSUM") as ssqpsum,
        ):
            xh_sb = cpool.tile([P, NT, BPC, C], dt.float8e4)
            xl_sb = cpool.tile([P, NT, BPC, C], dt.float8e4)
            oh_sb = cpool.tile([P, NT, BPC, C], dt.float8e4)
            ol_sb = cpool.tile([P, NT, BPC, C], dt.float8e4)
            ones_sb = cpool.tile([P, 2, P], dt.float8e4)
            scale_sb = cpool.tile([P, 2 * H], dt.float32)
            bias_sb = cpool.tile([P, NT], dt.float32)

            # x DMAs are issued inside the h==0 prologue below (after the
            # first two weight tiles) so the PE can start chunk-paced.
            NXSPLIT = 4
            nc.vector.memset(ones_sb[:], 1.0)

            def mm_evict(dst_ap, ps0b, ps1b):
                """walrus only allows one PSUM input per vector op.  The hi*hi
                psum (ps0) stops accumulating at the end of the FIRST pass, so
                evict it via Act as soon as it stops (frees its bank early and
                keeps the slot-reuse chain off the critical path); the DVE then
                combines the correction psum: dst = ps1/16 + p0s."""
                p0s = scrpool.tile([P, C], dt.bfloat16, tag="p1s", name="p0s")
                nc.scalar.activation(p0s[:], ps0b[:], AF.Copy)
                nc.vector.scalar_tensor_tensor(
                    out=dst_ap,
                    in0=ps1b[:],
                    scalar=1.0 / RSCALE,
                    in1=p0s[:],
                    op0=ALU.mult,
                    op1=ALU.add,
                )

            # kp pairs whose correction (cross) passes are skipped: spends a
            # measured slice of the error budget (err ~= 3.0e-2*sqrt(n/16/4
            # summed over groups)) to cut PE time by ~2*80ns*2b per tile per
            # dropped pair.  kp 0 and NP-1 must never be dropped (start/stop
            # flags live there).
            DROP = frozenset({7})

            def mm_passes(wh_sb, wl_sb, ps0, ps1, srch_sb, srcl_sb, kp_order=None,
                          drop=DROP):
                """Issue the three DoubleRow passes for one output tile:
                P0 += hi_x@hi_w;  P1 += lo_x@hi_w + hi_x@lo_w (both x16)."""
                if kp_order is None:
                    kp_order = list(range(NP))
                for i, kp in enumerate(kp_order):
                    ksl = slice(2 * kp, 2 * kp + 2)
                    for b in range(BPC):
                        nc.tensor.matmul(
                            ps0[b][:],
                            wh_sb[:, ksl, :],
                            srch_sb[:, ksl, b, :],
                            start=(i == 0),
                            stop=(i == NP - 1),
                            perf_mode=PM.DoubleRow,
                        )
                for i, kp in enumerate(kp_order):
                    if kp in drop:
                        continue
                    ksl = slice(2 * kp, 2 * kp + 2)
                    for b in range(BPC):
                        nc.tensor.matmul(
                            ps1[b][:],
                            wl_sb[:, ksl, :],
                            srch_sb[:, ksl, b, :],
                            start=(i == 0),
                            stop=False,
                            perf_mode=PM.DoubleRow,
                        )
                for i, kp in enumerate(kp_order):
                    if kp in drop:
                        continue
                    ksl = slice(2 * kp, 2 * kp + 2)
                    for b in range(BPC):
                        nc.tensor.matmul(
                            ps1[b][:],
                            wh_sb[:, ksl, :],
                            srcl_sb[:, ksl, b, :],
                            start=False,
                            stop=(i == NP - 1),
                            perf_mode=PM.DoubleRow,
                        )

            def alloc_ps(pool=None, tag="mm"):
                pool = pool or mmpsum
                ps0 = [
                    pool.tile([P, C], dt.float32, tag=tag, name="ps0")
                    for _ in range(BPC)
                ]
                ps1 = [
                    pool.tile([P, C], dt.float32, tag=tag, name="ps1")
                    for _ in range(BPC)
                ]
                return ps0, ps1

            def mm_tile(wh_src, wl_src, widx, dst, dst_sub, srch_sb, srcl_sb,
                        pspool=None):
                wh_sb = wpool.tile([P, NT, P], dt.float8e4, tag="w", name="wh_sb")
                nc.sync.dma_start(wh_sb[:], wh_src[widx])
                wl_sb = wpool.tile([P, NT, P], dt.float8e4, tag="w", name="wl_sb")
                nc.sync.dma_start(wl_sb[:], wl_src[widx])
                ps0, ps1 = alloc_ps(pspool, "mm" if pspool is None else "ssq")
                mm_passes(wh_sb, wl_sb, ps0, ps1, srch_sb, srcl_sb)
                for b in range(BPC):
                    mm_evict(dst[:, dst_sub, b, :], ps0[b], ps1[b])

            def issue_sq(q_sb, k_sb):
                """fp8 squares of q/k on the Act engine — one big op each.
                Issued right after the v1 tile so the Act engine computes them
                while the PE runs the v2/v3 tile groups."""
                sqs = {}
                for b in range(BPC):
                    for which, src in (("q", q_sb), ("k", k_sb)):
                        sq = sqpool.tile([P, MSUB, C], dt.float8e4, tag="sq", name="sq")
                        nc.scalar.activation(
                            sq[:], src[:, :, b, :], AF.Square, scale=1.0 / WSCALE
                        )
                        sqs[(b, which)] = sq
                return sqs

            def issue_ssq(h, sqs):
                """ssq ones-matmuls (PE DoubleRow) + rnorm = 1/||true||.
                rn_k additionally absorbs the folded per-head scale via the
                Sqrt input scale: sqrt(issq*s^2) = s/||k||."""
                rnorms = {}
                for b in range(BPC):
                    for which in ("q", "k"):
                        sq = sqs[(b, which)]
                        ssq_ps = ssqpsum.tile([P, C], dt.float32, tag="ssq", name="ssq_ps")
                        for sp in range(MSUB // 2):
                            nc.tensor.matmul(
                                ssq_ps[:],
                                ones_sb[:],
                                sq[:, 2 * sp : 2 * sp + 2, :],
                                start=(sp == 0),
                                stop=(sp == MSUB // 2 - 1),
                                perf_mode=PM.DoubleRow,
                            )
                        issq = nrmpool.tile([P, C], dt.float32, tag="issq", name="issq", bufs=1)
                        nc.vector.reciprocal(issq[:], ssq_ps[:])
                        rn = nrmpool.tile(
                            [P, C], dt.float32, tag=f"rn{which}{b}", name="rn", bufs=1
                        )
                        sc = scale_sb[:, H + h : H + h + 1] if which == "k" else 1.0
                        nc.scalar.activation(rn[:], issq[:], AF.Sqrt, scale=sc)
                        rnorms[(b, which)] = rn
                return rnorms

            def issue_kt_sub(b, sub, kt_sb, k_sb, v_sb, rnorms):
                """kt_sb[:, sub] = sum_c (v*rn_k)*k -- final after the fused
                multiply+reduce (scale folded into rn_k)."""
                vrk = scrpool.tile([P, C], dt.float32, tag="vrk", name="vrk", bufs=2)
                nc.vector.tensor_mul(vrk[:], v_sb[:, sub, b, :], rnorms[(b, "k")][:])
                prod = scrpool.tile([P, C], dt.float32, tag="prod", name="prod", bufs=2)
                nc.vector.tensor_mul(prod[:], vrk[:], k_sb[:, sub, b, :])
                nc.vector.tensor_reduce(
                    kt_sb[:, sub : sub + 1],
                    prod[:],
                    axis=mybir.AxisListType.X,
                    op=ALU.add,
                )

            def issue_kt(h, b, k_sb, v_sb, rnorms):
                kt_sb = nrmpool.tile([P, MSUB], dt.float32, tag="kt", name="kt_sb")
                for sub in range(MSUB):
                    issue_kt_sub(b, sub, kt_sb, k_sb, v_sb, rnorms)
                return kt_sb

            def issue_gelu(h, b, sub, q_sb, kt_sb, rnorms):
                """o = gelu(q_hat * kt), stored as fp8 pair at x64 scale."""
                gin = scrpool.tile([P, C], dt.float32, tag="gin", name="gin", bufs=2)
                nc.vector.scalar_tensor_tensor(
                    out=gin[:],
                    in0=q_sb[:, sub, b, :],
                    scalar=kt_sb[:, sub : sub + 1],
                    in1=rnorms[(b, "q")][:],
                    op0=ALU.mult,
                    op1=ALU.mult,
                )
                t_sb = scrpool.tile([P, C], dt.bfloat16, tag="t", name="t_sb")
                nc.scalar.activation(t_sb[:], gin[:], act_fn)
                osl = (slice(None), h * MSUB + sub, b, slice(None))
                nc.scalar.activation(
                    oh_sb[osl], t_sb[:], AF.Copy, scale=OSCALE
                )
                d_sb = scrpool.tile([P, C], dt.bfloat16, tag="d", name="d_sb")
                nc.vector.scalar_tensor_tensor(
                    out=d_sb[:],
                    in0=oh_sb[osl],
                    scalar=1.0 / OSCALE,
                    in1=t_sb[:],
                    op0=ALU.mult,
                    op1=ALU.subtract,
                )
                nc.vector.tensor_scalar_mul(
                    ol_sb[osl], d_sb[:], -OSCALE * RSCALE
                )

            def issue_kt_gelu(h, b, q_sb, k_sb, v_sb, rnorms):
                kt_sb = issue_kt(h, b, k_sb, v_sb, rnorms)
                for sub in range(MSUB):
                    issue_gelu(h, b, sub, q_sb, kt_sb, rnorms)

            # ---------------- stage 1: q/kv + attention + gelu ----------------
            for h in range(H):
                q_sb = qkvpool.tile([P, MSUB, BPC, C], dt.bfloat16, tag="q", name="q_sb")
                k_sb = qkvpool.tile([P, MSUB, BPC, C], dt.bfloat16, tag="k", name="k_sb")
                v_sb = qkvpool.tile([P, MSUB, BPC, C], dt.bfloat16, tag="v", name="v_sb")

                if h == 0:
                    # Prologue: start the PE as soon as w-tile 0/1 hi + the
                    # first x_hi chunk land.  Open the hi*hi groups of the
                    # first two q tiles (4 psums), then the correction group
                    # of tile 0 (2 more = all 6) — chunk-paced by the x DMAs.
                    w_ = NT // NXSPLIT

                    def xdma(dst, dsrc, xs):
                        nc.sync.dma_start(
                            dst[:, xs * w_ : (xs + 1) * w_],
                            dsrc[:, xs * w_ : (xs + 1) * w_],
                        )

                    whs, wls = [], []
                    xdma(xh_sb, xh_d, 0)
                    for t in range(2):
                        wh = wpool.tile([P, NT, P], dt.float8e4, tag="w", name="wh_sb")
                        nc.sync.dma_start(wh[:], wqh_d[t])
                        whs.append(wh)
                        wl = wpool.tile([P, NT, P], dt.float8e4, tag="w", name="wl_sb")
                        nc.sync.dma_start(wl[:], wql_d[t])
                        wls.append(wl)
                        xdma(xh_sb, xh_d, t + 1)
                    xdma(xh_sb, xh_d, 3)
                    for xs in range(NXSPLIT):
                        xdma(xl_sb, xl_d, xs)
                    nc.sync.dma_start(scale_sb[:], scale_d[:])
                    nc.sync.dma_start(bias_sb[:], bias_d[:])
                    # 4 of 6 psum bufs for the two hi*hi groups; the t0
                    # correction group takes the last 2, t1's waits for t0's
                    # eviction to free slots (no deadlock).
                    ps0s = [
                        [
                            mmpsum.tile([P, C], dt.float32, tag="mm", name="ps0")
                            for _ in range(BPC)
                        ]
                        for _ in range(2)
                    ]
                    # interleave the two hi*hi groups at x-chunk granularity
                    CHUNK = NP // NXSPLIT
                    for xs in range(NXSPLIT):
                        for t in range(2):
                            for kp in range(xs * CHUNK, (xs + 1) * CHUNK):
                                ksl = slice(2 * kp, 2 * kp + 2)
                                for b in range(BPC):
                                    nc.tensor.matmul(
                                        ps0s[t][b][:],
                                        whs[t][:, ksl, :],
                                        xh_sb[:, ksl, b, :],
                                        start=(kp == 0),
                                        stop=(kp == NP - 1),
                                        perf_mode=PM.DoubleRow,
                                    )
                    ps1s = [
                        [
                            pool.tile([P, C], dt.float32, tag=tg, name="ps1")
                            for _ in range(BPC)
                        ]
                        for pool, tg in ((mmpsum, "mm"), (ssqpsum, "ssq"))
                    ]
                    # wl cross-pass for both tiles (dense work while xl loads)
                    for t in range(2):
                        for i, kp in enumerate(range(NP)):
                            ksl = slice(2 * kp, 2 * kp + 2)
                            for b in range(BPC):
                                nc.tensor.matmul(
                                    ps1s[t][b][:],
                                    wls[t][:, ksl, :],
                                    xh_sb[:, ksl, b, :],
                                    start=(i == 0),
                                    stop=False,
                                    perf_mode=PM.DoubleRow,
                                )
                    # xl cross-pass, both tiles interleaved at chunk pace
                    for xs in range(NXSPLIT):
                        for t in range(2):
                            for kp in range(xs * CHUNK, (xs + 1) * CHUNK):
                                ksl = slice(2 * kp, 2 * kp + 2)
                                for b in range(BPC):
                                    nc.tensor.matmul(
                                        ps1s[t][b][:],
                                        whs[t][:, ksl, :],
                                        xl_sb[:, ksl, b, :],
                                        start=False,
                                        stop=(kp == NP - 1),
                                        perf_mode=PM.DoubleRow,
                                    )
                    for t in range(2):
                        for b in range(BPC):
                            mm_evict(q_sb[:, t, b, :], ps0s[t][b], ps1s[t][b])
                    first_q = 2
                else:
                    first_q = 0

                for sub in range(first_q, MSUB):
                    mm_tile(wqh_d, wql_d, h * MSUB + sub, q_sb, sub, xh_sb, xl_sb)
                for sub in range(MSUB):
                    mm_tile(wkvh_d, wkvl_d, h * MSUB + sub, k_sb, sub, xh_sb, xl_sb)
                # sq on Act during the v0/v1 tile groups; ssq matmuls issued
                # after v1 so the PE reaches them with sq long since done.
                sqs = issue_sq(q_sb, k_sb)
                for sub in range(2):
                    mm_tile(wkvh_d, wkvl_d, NT + h * MSUB + sub, v_sb, sub, xh_sb, xl_sb)
                rnorms = issue_ssq(h, sqs)
                for sub in range(2, MSUB):
                    mm_tile(wkvh_d, wkvl_d, NT + h * MSUB + sub, v_sb, sub, xh_sb, xl_sb)
                if h < H - 1:
                    for b in range(BPC):
                        issue_kt_gelu(h, b, q_sb, k_sb, v_sb, rnorms)
                else:
                    kts = [
                        nrmpool.tile([P, MSUB], dt.float32, tag="kt", name="kt_sb")
                        for _ in range(BPC)
                    ]
                    for sub in range(MSUB):
                        for b in range(BPC):
                            issue_kt_sub(b, sub, kts[b], k_sb, v_sb, rnorms)
                            issue_gelu(h, b, sub, q_sb, kts[b], rnorms)

            # ---------------- stage 2: output projection ----------------
            # kp pairs 14/15 (= head 7's rows) are issued after all three
            # passes' earlier pairs, so the first tiles don't stall on the
            # last head's gelu outputs still in flight.
            for mt in range(NT):
                wh_sb = wpool.tile([P, NT, P], dt.float8e4, tag="w", name="wh_sb")
                nc.sync.dma_start(wh_sb[:], wph_d[mt])
                wl_sb = wpool.tile([P, NT, P], dt.float8e4, tag="w", name="wl_sb")
                nc.sync.dma_start(wl_sb[:], wpl_d[mt])
                ps0, ps1 = alloc_ps()
                for kps in (range(NP - 4), range(NP - 4, NP)):
                    for kp in kps:
                        ksl = slice(2 * kp, 2 * kp + 2)
                        for b in range(BPC):
                            nc.tensor.matmul(
                                ps0[b][:],
                                wh_sb[:, ksl, :],
                                oh_sb[:, ksl, b, :],
                                start=(kp == 0),
                                stop=(kp == NP - 1),
                                perf_mode=PM.DoubleRow,
                            )
                    for kp in kps:
                        if kp in DROP:
                            continue
                        ksl = slice(2 * kp, 2 * kp + 2)
                        for b in range(BPC):
                            nc.tensor.matmul(
                                ps1[b][:],
                                wh_sb[:, ksl, :],
                                ol_sb[:, ksl, b, :],
                                start=(kp == 0),
                                stop=False,
                                perf_mode=PM.DoubleRow,
                            )
                    for kp in kps:
                        if kp in DROP:
                            continue
                        ksl = slice(2 * kp, 2 * kp + 2)
                        for b in range(BPC):
                            nc.tensor.matmul(
                                ps1[b][:],
                                wl_sb[:, ksl, :],
                                oh_sb[:, ksl, b, :],
                                start=False,
                                stop=(kp == NP - 1),
                                perf_mode=PM.DoubleRow,
                            )
                for b in range(BPC):
                    # p0s = ps0/4096 + bias (early: ps0 stops after pass 1);
                    # ob = ps1/(16*4096) + p0s
                    p0s = outpool.tile([P, C], dt.float32, tag="p1s2", name="p0s")
                    nc.scalar.activation(
                        p0s[:], ps0[b][:], AF.Identity,
                        scale=1.0 / (WSCALE * OSCALE),
                        bias=bias_sb[:, mt : mt + 1],
                    )
                    ob = outpool.tile([P, C], dt.float32, tag="ob", name="ob")
                    nc.vector.scalar_tensor_tensor(
                        out=ob[:],
                        in0=ps1[b][:],
                        scalar=1.0 / (RSCALE * WSCALE * OSCALE),
                        in1=p0s[:],
                        op0=ALU.mult,
                        op1=ALU.add,
                    )
                    nc.sync.dma_start(out_d[b, mt], ob[:])

    nc.compile()
    return nc


def _split_fp8(a):
    """a (fp32, ~N(0,1) scale) -> (hi, lo) fp8 with a ~= hi + lo/16."""
    hi = a.astype(F8)
    lo = ((a - hi.astype(np.float32)) * RSCALE).astype(F8)
    return hi, lo


def _prep_inputs(x, wq, wkv, wproj, bproj, scale):
    x = np.asarray(x, dtype=np.float32)
    wq = np.asarray(wq, dtype=np.float32)
    wkv = np.asarray(wkv, dtype=np.float32)
    wproj = np.asarray(wproj, dtype=np.float32)
    bproj = np.asarray(bproj, dtype=np.float32)
    scale = np.asarray(scale, dtype=np.float32)

    # W[mt, p, nt, j] = w[mt*128 + j, nt*128 + p]  (transposed tile layout),
    # scaled x64 then split into fp8 hi/lo pairs.
    def wtiles(w, mtiles):
        wt = np.ascontiguousarray(
            w.reshape(mtiles, P, NT, P).transpose(0, 3, 2, 1)
        ) * WSCALE
        return _split_fp8(wt)

    Wqh, Wql = wtiles(wq, NT)
    Wkvh, Wkvl = wtiles(wkv, 2 * NT)
    Wph, Wpl = wtiles(wproj, NT)

    # X[core][p, nt, b, c] = x[2*core + b, nt*128 + p, c]
    Xall = np.ascontiguousarray(
        x.reshape(NCORES, BPC, NT, P, C).transpose(0, 3, 2, 1, 4)
    )
    Xh, Xl = _split_fp8(Xall)

    # stage-2 eviction: p1s2 = ps1/(16*4096) + bias; ob = ps0/4096 + p1s2
    bias = np.ascontiguousarray(bproj.reshape(NT, P).T)
    # kt folding: kt_raw = 4096 * sum(khat*v); want kt = kt_true*scale/64.
    # The factor scale/64^3 is folded into rn_k as sqrt(issq * s^2), so
    # column H+h carries s^2.
    s1 = scale.reshape(1, H) / (WSCALE * WSCALE * WSCALE)
    scale_b = np.ascontiguousarray(
        np.broadcast_to(np.concatenate([s1, s1 * s1], axis=1), (P, 2 * H))
    ).astype(np.float32)

    in_maps = []
    for c in range(NCORES):
        in_maps.append(
            {
                "x_hi": Xh[c],
                "x_lo": Xl[c],
                "wq_hi": Wqh,
                "wq_lo": Wql,
                "wkv_hi": Wkvh,
                "wkv_lo": Wkvl,
                "wproj_hi": Wph,
                "wproj_lo": Wpl,
                "bias": bias,
                "scale": scale_b,
            }
        )
    return in_maps


def kernel(x, wq, wkv, wproj, bproj, scale):
    global _NC, _LAST_RESULTS
    if _NC is None:
        _NC = _build_nc()

    in_maps = _prep_inputs(x, wq, wkv, wproj, bproj, scale)
    res = run_bass_kernel_spmd(_NC, in_maps, core_ids=list(range(NCORES)))
    _LAST_RESULTS = res

    outs = [res.results[c]["out"].reshape(BPC, N, C) for c in range(NCORES)]
    return np.ascontiguousarray(np.concatenate(outs, axis=0), dtype=np.float32)
